# revision 33
# baseline (speedup 1.0000x reference)
"""Gabor-atom synthesis via exact Fourier-basis factorization, time-sharded
across 8 cores.

Each 120-sample chunk of the output is synthesized in a fixed 128-function
Fourier basis (cos/sin at bin spacing 2*pi/128 per sample, columns
interleaved by bin so each atom's spectrum is contiguous). Atoms are sorted
by carrier frequency into 16 blocks of 128; for most blocks the per-atom
coefficient vector is solved (windowed LS) against only a 64-column aligned
half of the basis, halving the stationary-matrix DMA with no extra matmuls.
The chirp (gamma) drifts beta by ~1e-6 rad/sample over the whole signal, so
one coefficient vector per atom serves every chunk; the per-chunk phase
alpha is computed exactly on the host and folded into the envelope columns.

Device pipeline (per core, 50 chunks in 4 column-range pieces):
  stage1 (PE): per atom-block, contract [128 atoms] x per-chunk complex
    amplitudes into per-piece PSUM bins (PSUM pre-zeroed, start=False);
    envelope is piecewise-linear per chunk (e0 cols f16, delta cols
    fp8-e4m3, packed in one u16 DMA tensor per piece).
  stage2 (PE): one pair of matmuls, Z (f16) stationary against the basis
    matrix (moving) -> chunk-major [50, 128] PSUM with the envelope ramp
    folded into the second basis matrix.
  copies: Z PSUM->SBUF per piece on DVE, Y PSUM->SBUF on ACT.
Dummy PE matmuls pace the idle DMA window so the pstate ramp reaches full
clock before the real matmuls run. No per-sample elementwise work.
"""
import numpy as np
import ml_dtypes
from contextlib import ExitStack

import concourse.bacc as bacc
import concourse.tile as tile
from concourse import mybir
from concourse.bass_utils import run_bass_kernel_spmd

FS = 24000.0
T = 48000
N_ATOMS = 2048
N_CORES = 8
NYQUIST = FS / 2.0
SIGMA_OFFSET = 1e-3

P = 128                      # partitions / atoms per block
NB = 16                      # atom blocks
S = 120                      # samples per chunk
SP = 128                     # padded output row length
F = 128                      # basis functions
TC = T // N_CORES            # 6000 samples per core
NCH = TC // S                # 50 chunks per core
PCS = (16, 14, 12, 8)        # chunk-range piece sizes (even, decreasing)
SD = 64.0                    # fp8 scale for envelope-delta columns
MARG = 6                     # min window margin (bins)

f32 = mybir.dt.float32
f16 = mybir.dt.float16
u16 = mybir.dt.uint16
f8 = mybir.dt.float8e4

_cache = {}


def _basis():
    """Interleaved-column basis: col0=cos_0, col(2k-1)=cos_k, col(2k)=sin_k
    for k=1..63, col127=cos_64. An atom at bin b has its energy near column
    2b, so a block's spectrum fits one aligned 64-column window."""
    i_ = np.arange(S)
    B = np.zeros((S, F))
    B[:, 0] = 1.0
    for k in range(1, 64):
        B[:, 2 * k - 1] = np.cos(2 * np.pi * i_ * k / F)
        B[:, 2 * k] = np.sin(2 * np.pi * i_ * k / F)
    B[:, 127] = np.cos(2 * np.pi * i_ * 64 / F)
    return B


_B = _basis()


def _build_program(wins):
    """wins: tuple of (lo, hi) basis-column windows per block."""
    nc = bacc.Bacc("TRN2", target_bir_lowering=False, debug=False)

    kw = [2 * (hi - lo) for lo, hi in wins]          # P+Q cols per block
    koff = np.concatenate([[0], np.cumsum(kw)]).astype(int)
    KW = int(koff[-1])

    d_k = nc.dram_tensor("kmat", [P, KW], f16, kind="ExternalInput").ap()
    d_e = [nc.dram_tensor(f"emat{p}", [P, NB * 3 * nch], u16,
                          kind="ExternalInput").ap()
           for p, nch in enumerate(PCS)]
    d_b = nc.dram_tensor("bmat", [P, 2 * SP], f16, kind="ExternalInput").ap()
    d_out = nc.dram_tensor("wave", [P, NCH], f32, kind="ExternalOutput").ap()

    offs = np.concatenate([[0], np.cumsum(PCS)]).astype(int)
    NP = len(PCS)

    with tile.TileContext(nc) as tc, ExitStack() as ctx:
        consts = ctx.enter_context(tc.tile_pool(name="consts", bufs=1))
        kpool = ctx.enter_context(tc.tile_pool(name="kp", bufs=1))
        epool = ctx.enter_context(tc.tile_pool(name="ep", bufs=1))
        zpool = ctx.enter_context(tc.tile_pool(name="zp", bufs=1))
        opool = ctx.enter_context(tc.tile_pool(name="op", bufs=1))
        rpool = ctx.enter_context(tc.tile_pool(name="rp", bufs=1, space="PSUM"))

        # PE pstate pacing: dummy matmuls keep PE busy from ~1us until the
        # first real matmul (~E0 landed) so the clock ramps to full speed.
        t_wm = consts.tile([P, P], f16)
        nc.vector.memset(t_wm[:], 0.0)
        p_warm = rpool.tile([16, P], f32)

        p_z = [rpool.tile([P, 2 * nch], f32, name=f"z{p}")
               for p, nch in enumerate(PCS)]
        for p in range(NP):
            nc.vector.memset(p_z[p][:], 0.0)


        t_k = kpool.tile([P, KW], f16)
        t_e = [epool.tile([P, NB * 3 * nch], u16, name=f"e{p}")
               for p, nch in enumerate(PCS)]
        t_b = consts.tile([P, 2 * SP], f16)
        nc.sync.dma_start(t_k[:], d_k[:])
        for p in range(NP):
            nc.sync.dma_start(t_e[p][:], d_e[p][:])
        nc.sync.dma_start(t_b[:], d_b[:])

        # pace dummies: engine model 1/0.65GHz -> 1/1.2 -> 1/2.4 per col
        t0 = 1000.0
        target = 666 + 650 + 8 * (KW * 2 + NB * 3 * PCS[0] * 2) / 22.5 + 900
        t = t0
        nd = 0
        while t < target:
            ramp = t - t0
            cyc = 1.538 if ramp < 100 else (0.833 if ramp < 3000 else 0.417)
            nc.tensor.matmul(p_warm[:16, :], t_wm[:, 0:16], t_wm[:],
                             start=True, stop=True)
            t += P * cyc
            nd += 1


        def stage1(p):
            nch = PCS[p]
            ef = t_e[p][:].bitcast(f16)
            e8 = t_e[p][:].bitcast(f8)
            for g in range(NB):
                lo, hi = wins[g]
                ko = int(koff[g])
                w = hi - lo
                eb = g * 3 * nch
                eb8 = 2 * (g * 3 * nch + 2 * nch)
                last = (g == NB - 1)
                nc.tensor.matmul(p_z[p][lo:hi, 0:nch],
                                 t_k[:, ko:ko + w],
                                 ef[:, eb:eb + nch],
                                 start=False, stop=False)
                nc.tensor.matmul(p_z[p][lo:hi, 0:nch],
                                 t_k[:, ko + w:ko + 2 * w],
                                 ef[:, eb + nch:eb + 2 * nch],
                                 start=False, stop=False)
                nc.tensor.matmul(p_z[p][lo:hi, nch:2 * nch],
                                 t_k[:, ko:ko + w],
                                 e8[:, eb8:eb8 + nch],
                                 start=False, stop=False)
                nc.tensor.matmul(p_z[p][lo:hi, nch:2 * nch],
                                 t_k[:, ko + w:ko + 2 * w],
                                 e8[:, eb8 + nch:eb8 + 2 * nch],
                                 start=False, stop=last)

        # piece-major Z staging: one PSUM->SBUF copy per piece, stage2 runs
        # per piece into disjoint PSUM column ranges (same total PE columns).
        # Output Y PSUM is split in two tiles so the big early output DMA
        # (chunks 0..CS) has no dependency on the last piece, and its fixed
        # HWDGE/DGE chain overlaps the small final DMA's compute.
        CS = int(offs[NP - 1])
        CL = NCH - CS
        t_z = [zpool.tile([P, 2 * nch], f16, name=f"tz{p}")
               for p, nch in enumerate(PCS)]
        t_ya = opool.tile([P, CS], f32, name="tya")
        t_yb = opool.tile([P, CL], f32, name="tyb")
        p_ya = rpool.tile([P, CS], f32, name="pya")
        p_yb = rpool.tile([P, CL], f32, name="pyb")

        def zcopy(p):
            eng = nc.vector.tensor_copy if p % 2 == 0 else (
                lambda d, s: nc.scalar.activation(
                    d, s, mybir.ActivationFunctionType.Copy))
            eng(t_z[p][:], p_z[p][:])

        def stage2(p):
            nch = PCS[p]
            c0 = int(offs[p])
            py, yo = (p_ya, c0) if p < NP - 1 else (p_yb, 0)
            nc.tensor.matmul(py[:, yo:yo + nch], t_b[:, 0:SP],
                             t_z[p][:, 0:nch], start=True, stop=False)
            nc.tensor.matmul(py[:, yo:yo + nch], t_b[:, SP:2 * SP],
                             t_z[p][:, nch:2 * nch], start=False, stop=True)

        stage1(0)
        zcopy(0)
        for p in range(1, NP):
            stage1(p)
            stage2(p - 1)
            zcopy(p)
        nc.vector.tensor_copy(t_ya[:], p_ya[:])
        nc.sync.dma_start(d_out[:, 0:CS], t_ya[:])
        stage2(NP - 1)
        nc.scalar.activation(t_yb[:], p_yb[:],
                             mybir.ActivationFunctionType.Copy)
        nc.sync.dma_start(d_out[:, CS:NCH], t_yb[:])

    nc.compile()
    return nc


def _windows(beta_sorted):
    """Per-block aligned basis-column window: [0,64), [64,128) or full."""
    wins = []
    for g in range(NB):
        kc = beta_sorted[g * P:(g + 1) * P] * F / (2 * np.pi)
        if kc.max() + MARG <= 31.5:
            wins.append((0, 64))
        elif kc.min() - MARG >= 31.5:
            wins.append((64, 128))
        else:
            wins.append((0, 128))
    return tuple(wins)


def _prepare(amplitude_logit, tau, omega_logit, sigma_logit, phi_vector, gamma):
    al = amplitude_logit.astype(np.float64)
    tau = tau.astype(np.float64)
    ol = omega_logit.astype(np.float64)
    sl = sigma_logit.astype(np.float64)
    pv = phi_vector.astype(np.float64)
    gam = gamma.astype(np.float64)

    amp = np.where(al > 30, al, np.log1p(np.exp(al)))
    omega = (1.0 / (1.0 + np.exp(-ol))) * 0.99 * NYQUIST
    sigma = np.where(sl > 30, sl, np.log1p(np.exp(sl))) + SIGMA_OFFSET
    phi = np.arctan2(pv[:, 1], pv[:, 0])

    # per-atom frequency (rad/sample) at signal center; chirp drift over the
    # full 2 s is ~4e-6 rad/sample -> phase error < 5e-4 within any chunk
    beta = (2 * np.pi * omega + 2 * gam * (1.0 - tau)) / FS

    order = np.argsort(beta, kind="stable")
    beta, amp, tau, sigma, omega, gam, phi = (
        x[order] for x in (beta, amp, tau, sigma, omega, gam, phi))

    wins = _windows(beta)
    i_ = np.arange(S)

    kw = [2 * (hi - lo) for lo, hi in wins]
    koff = np.concatenate([[0], np.cumsum(kw)]).astype(int)
    kmat = np.empty((P, int(koff[-1])), np.float16)
    for g in range(NB):
        sel = slice(g * P, (g + 1) * P)
        lo, hi = wins[g]
        w = hi - lo
        pinv = np.linalg.pinv(_B[:, lo:hi])              # [w, S]
        C = np.cos(np.outer(beta[sel], i_))
        Sn = np.sin(np.outer(beta[sel], i_))
        ko = int(koff[g])
        kmat[:, ko:ko + w] = (C @ pinv.T).astype(np.float16)
        kmat[:, ko + w:ko + 2 * w] = (-(Sn @ pinv.T)).astype(np.float16)

    bmat = np.zeros((P, 2 * SP), np.float16)
    bmat[:, 0:S] = _B.T.astype(np.float16)                      # B[f, i]
    bmat[:, SP:SP + S] = ((_B * (i_ / S)[:, None]).T / SD).astype(np.float16)

    # envelope at all global chunk nodes and phase at all chunk starts
    nodes = np.arange(N_CORES * NCH + 1) * S / FS               # [401]
    G = amp[:, None] * np.exp(
        -0.5 * ((nodes[None, :] - tau[:, None]) / sigma[:, None]) ** 2)
    starts = np.arange(N_CORES * NCH) * S / FS                  # [400]
    dt = starts[None, :] - tau[:, None]
    ph = 2 * np.pi * omega[:, None] * dt + gam[:, None] * dt * dt + phi[:, None]
    ca = np.cos(ph)
    sa = np.sin(ph)
    e0 = G[:, :-1]
    de = G[:, 1:] - G[:, :-1]
    Ec0 = (e0 * ca).astype(np.float16).view(np.uint16)
    Es0 = (e0 * sa).astype(np.float16).view(np.uint16)
    Dc = (de * ca * SD).astype(ml_dtypes.float8_e4m3fn).view(np.uint8)
    Ds = (de * sa * SD).astype(ml_dtypes.float8_e4m3fn).view(np.uint8)

    offs = np.concatenate([[0], np.cumsum(PCS)]).astype(int)
    in_maps = []
    for c in range(N_CORES):
        m = {"kmat": kmat, "bmat": bmat}
        for p, nch in enumerate(PCS):
            cs = slice(c * NCH + int(offs[p]), c * NCH + int(offs[p]) + nch)
            em = np.empty((P, NB * 3 * nch), np.uint16)
            for g in range(NB):
                sel = slice(g * P, (g + 1) * P)
                o = g * 3 * nch
                em[:, o:o + nch] = Ec0[sel, cs]
                em[:, o + nch:o + 2 * nch] = Es0[sel, cs]
                pk = np.concatenate([Dc[sel, cs], Ds[sel, cs]], axis=1)
                em[:, o + 2 * nch:o + 3 * nch] = \
                    np.ascontiguousarray(pk).view(np.uint16)
            m[f"emat{p}"] = np.ascontiguousarray(em)
        in_maps.append(m)
    return wins, in_maps


def kernel(amplitude_logit, tau, omega_logit, sigma_logit, phi_vector, gamma, t):
    wins, in_maps = _prepare(amplitude_logit, tau, omega_logit, sigma_logit,
                             phi_vector, gamma)
    if wins not in _cache:
        _cache[wins] = _build_program(wins)
    nc = _cache[wins]
    res = run_bass_kernel_spmd(nc, in_maps, list(range(N_CORES)))
    total = np.empty(T, dtype=np.float32)
    for c, r in enumerate(res.results):
        wv = r["wave"].astype(np.float32)          # [128, NCH]
        total[c * TC:(c + 1) * TC] = wv[:S].T.ravel()
    return total


# revision 34
# speedup vs baseline: 1.0001x; 1.0001x over previous
"""Gabor-atom synthesis via exact Fourier-basis factorization, time-sharded
across 8 cores.

Each 120-sample chunk of the output is synthesized in a fixed 128-function
Fourier basis (cos/sin at bin spacing 2*pi/128 per sample, columns
interleaved by bin so each atom's spectrum is contiguous). Atoms are sorted
by carrier frequency into 16 blocks of 128; for most blocks the per-atom
coefficient vector is solved (windowed LS) against only a 64-column aligned
half of the basis, halving the stationary-matrix DMA with no extra matmuls.
The chirp (gamma) drifts beta by ~1e-6 rad/sample over the whole signal, so
one coefficient vector per atom serves every chunk; the per-chunk phase
alpha is computed exactly on the host and folded into the envelope columns.

Device pipeline (per core, 50 chunks in 4 column-range pieces):
  stage1 (PE): per atom-block, contract [128 atoms] x per-chunk complex
    amplitudes into per-piece PSUM bins (PSUM pre-zeroed, start=False);
    envelope is piecewise-linear per chunk (e0 cols f16, delta cols
    fp8-e4m3, packed in one u16 DMA tensor per piece).
  stage2 (PE): one pair of matmuls, Z (f16) stationary against the basis
    matrix (moving) -> chunk-major [50, 128] PSUM with the envelope ramp
    folded into the second basis matrix.
  copies: Z PSUM->SBUF per piece on DVE, Y PSUM->SBUF on ACT.
Dummy PE matmuls pace the idle DMA window so the pstate ramp reaches full
clock before the real matmuls run. No per-sample elementwise work.
"""
import numpy as np
import ml_dtypes
from contextlib import ExitStack

import concourse.bacc as bacc
import concourse.tile as tile
from concourse import mybir
from concourse.bass_utils import run_bass_kernel_spmd

FS = 24000.0
T = 48000
N_ATOMS = 2048
N_CORES = 8
NYQUIST = FS / 2.0
SIGMA_OFFSET = 1e-3

P = 128                      # partitions / atoms per block
NB = 16                      # atom blocks
S = 120                      # samples per chunk
SP = 128                     # padded output row length
F = 128                      # basis functions
TC = T // N_CORES            # 6000 samples per core
NCH = TC // S                # 50 chunks per core
PCS = (16, 14, 12, 8)        # chunk-range piece sizes (even, decreasing)
SD = 64.0                    # fp8 scale for envelope-delta columns
MARG = 6                     # min window margin (bins)

f32 = mybir.dt.float32
f16 = mybir.dt.float16
u16 = mybir.dt.uint16
f8 = mybir.dt.float8e4

_cache = {}


def _basis():
    """Interleaved-column basis: col0=cos_0, col(2k-1)=cos_k, col(2k)=sin_k
    for k=1..63, col127=cos_64. An atom at bin b has its energy near column
    2b, so a block's spectrum fits one aligned 64-column window."""
    i_ = np.arange(S)
    B = np.zeros((S, F))
    B[:, 0] = 1.0
    for k in range(1, 64):
        B[:, 2 * k - 1] = np.cos(2 * np.pi * i_ * k / F)
        B[:, 2 * k] = np.sin(2 * np.pi * i_ * k / F)
    B[:, 127] = np.cos(2 * np.pi * i_ * 64 / F)
    return B


_B = _basis()


def _build_program(wins):
    """wins: tuple of (lo, hi) basis-column windows per block."""
    nc = bacc.Bacc("TRN2", target_bir_lowering=False, debug=False)

    kw = [2 * (hi - lo) for lo, hi in wins]          # P+Q cols per block
    koff = np.concatenate([[0], np.cumsum(kw)]).astype(int)
    KW = int(koff[-1])

    d_k = nc.dram_tensor("kmat", [P, KW], f16, kind="ExternalInput").ap()
    d_e = [nc.dram_tensor(f"emat{p}", [P, NB * 3 * nch], u16,
                          kind="ExternalInput").ap()
           for p, nch in enumerate(PCS)]
    d_b = nc.dram_tensor("bmat", [P, 2 * SP], f16, kind="ExternalInput").ap()
    d_out = nc.dram_tensor("wave", [P, NCH], f32, kind="ExternalOutput").ap()

    offs = np.concatenate([[0], np.cumsum(PCS)]).astype(int)
    NP = len(PCS)

    with tile.TileContext(nc) as tc, ExitStack() as ctx:
        consts = ctx.enter_context(tc.tile_pool(name="consts", bufs=1))
        kpool = ctx.enter_context(tc.tile_pool(name="kp", bufs=1))
        epool = ctx.enter_context(tc.tile_pool(name="ep", bufs=1))
        zpool = ctx.enter_context(tc.tile_pool(name="zp", bufs=1))
        opool = ctx.enter_context(tc.tile_pool(name="op", bufs=1))
        rpool = ctx.enter_context(tc.tile_pool(name="rp", bufs=1, space="PSUM"))

        # PE pstate pacing: dummy matmuls keep PE busy from ~1us until the
        # first real matmul (~E0 landed) so the clock ramps to full speed.
        t_wm = consts.tile([P, P], f16)
        nc.vector.memset(t_wm[:], 0.0)
        p_warm = rpool.tile([16, P], f32)

        p_z = [rpool.tile([P, 2 * nch], f32, name=f"z{p}")
               for p, nch in enumerate(PCS)]
        for p in range(NP):
            nc.vector.memset(p_z[p][:], 0.0)


        t_k = kpool.tile([P, KW], f16)
        t_e = [epool.tile([P, NB * 3 * nch], u16, name=f"e{p}")
               for p, nch in enumerate(PCS)]
        t_b = consts.tile([P, 2 * SP], f16)
        nc.sync.dma_start(t_k[:], d_k[:])
        for p in range(NP):
            nc.sync.dma_start(t_e[p][:], d_e[p][:])
        nc.sync.dma_start(t_b[:], d_b[:])

        # pace dummies: engine model 1/0.65GHz -> 1/1.2 -> 1/2.4 per col
        t0 = 1000.0
        target = 666 + 650 + 8 * (KW * 2 + NB * 3 * PCS[0] * 2) / 22.5 + 900
        t = t0
        nd = 0
        while t < target:
            ramp = t - t0
            cyc = 1.538 if ramp < 100 else (0.833 if ramp < 3000 else 0.417)
            nc.tensor.matmul(p_warm[:16, :], t_wm[:, 0:16], t_wm[:],
                             start=True, stop=True)
            t += P * cyc
            nd += 1


        def stage1(p):
            nch = PCS[p]
            ef = t_e[p][:].bitcast(f16)
            e8 = t_e[p][:].bitcast(f8)
            for g in range(NB):
                lo, hi = wins[g]
                ko = int(koff[g])
                w = hi - lo
                eb = g * 3 * nch
                eb8 = 2 * (g * 3 * nch + 2 * nch)
                last = (g == NB - 1)
                nc.tensor.matmul(p_z[p][lo:hi, 0:nch],
                                 t_k[:, ko:ko + w],
                                 ef[:, eb:eb + nch],
                                 start=False, stop=False)
                nc.tensor.matmul(p_z[p][lo:hi, 0:nch],
                                 t_k[:, ko + w:ko + 2 * w],
                                 ef[:, eb + nch:eb + 2 * nch],
                                 start=False, stop=False)
                nc.tensor.matmul(p_z[p][lo:hi, nch:2 * nch],
                                 t_k[:, ko:ko + w],
                                 e8[:, eb8:eb8 + nch],
                                 start=False, stop=False)
                nc.tensor.matmul(p_z[p][lo:hi, nch:2 * nch],
                                 t_k[:, ko + w:ko + 2 * w],
                                 e8[:, eb8 + nch:eb8 + 2 * nch],
                                 start=False, stop=last)

        # piece-major Z staging: one PSUM->SBUF copy per piece, stage2 runs
        # per piece into disjoint PSUM column ranges (same total PE columns).
        # Output Y PSUM is split in two tiles so the big early output DMA
        # (chunks 0..CS) has no dependency on the last piece, and its fixed
        # HWDGE/DGE chain overlaps the small final DMA's compute.
        CS = int(offs[NP - 2])
        CL = NCH - CS
        t_z = [zpool.tile([P, 2 * nch], f16, name=f"tz{p}")
               for p, nch in enumerate(PCS)]
        t_ya = opool.tile([P, CS], f32, name="tya")
        t_yb = opool.tile([P, CL], f32, name="tyb")
        p_ya = rpool.tile([P, CS], f32, name="pya")
        p_yb = rpool.tile([P, CL], f32, name="pyb")

        def zcopy(p):
            eng = nc.vector.tensor_copy if p % 2 == 0 else (
                lambda d, s: nc.scalar.activation(
                    d, s, mybir.ActivationFunctionType.Copy))
            eng(t_z[p][:], p_z[p][:])

        def stage2(p):
            nch = PCS[p]
            c0 = int(offs[p])
            py, yo = (p_ya, c0) if p < NP - 2 else (p_yb, c0 - CS)
            nc.tensor.matmul(py[:, yo:yo + nch], t_b[:, 0:SP],
                             t_z[p][:, 0:nch], start=True, stop=False)
            nc.tensor.matmul(py[:, yo:yo + nch], t_b[:, SP:2 * SP],
                             t_z[p][:, nch:2 * nch], start=False, stop=True)

        stage1(0)
        zcopy(0)
        for p in range(1, NP):
            stage1(p)
            stage2(p - 1)
            if p == NP - 1:
                nc.vector.tensor_copy(t_ya[:], p_ya[:])
                nc.sync.dma_start(d_out[:, 0:CS], t_ya[:])
            zcopy(p)
        stage2(NP - 1)
        nc.scalar.activation(t_yb[:], p_yb[:],
                             mybir.ActivationFunctionType.Copy)
        nc.sync.dma_start(d_out[:, CS:NCH], t_yb[:])

    nc.compile()
    return nc


def _windows(beta_sorted):
    """Per-block aligned basis-column window: [0,64), [64,128) or full."""
    wins = []
    for g in range(NB):
        kc = beta_sorted[g * P:(g + 1) * P] * F / (2 * np.pi)
        if kc.max() + MARG <= 31.5:
            wins.append((0, 64))
        elif kc.min() - MARG >= 31.5:
            wins.append((64, 128))
        else:
            wins.append((0, 128))
    return tuple(wins)


def _prepare(amplitude_logit, tau, omega_logit, sigma_logit, phi_vector, gamma):
    al = amplitude_logit.astype(np.float64)
    tau = tau.astype(np.float64)
    ol = omega_logit.astype(np.float64)
    sl = sigma_logit.astype(np.float64)
    pv = phi_vector.astype(np.float64)
    gam = gamma.astype(np.float64)

    amp = np.where(al > 30, al, np.log1p(np.exp(al)))
    omega = (1.0 / (1.0 + np.exp(-ol))) * 0.99 * NYQUIST
    sigma = np.where(sl > 30, sl, np.log1p(np.exp(sl))) + SIGMA_OFFSET
    phi = np.arctan2(pv[:, 1], pv[:, 0])

    # per-atom frequency (rad/sample) at signal center; chirp drift over the
    # full 2 s is ~4e-6 rad/sample -> phase error < 5e-4 within any chunk
    beta = (2 * np.pi * omega + 2 * gam * (1.0 - tau)) / FS

    order = np.argsort(beta, kind="stable")
    beta, amp, tau, sigma, omega, gam, phi = (
        x[order] for x in (beta, amp, tau, sigma, omega, gam, phi))

    wins = _windows(beta)
    i_ = np.arange(S)

    kw = [2 * (hi - lo) for lo, hi in wins]
    koff = np.concatenate([[0], np.cumsum(kw)]).astype(int)
    kmat = np.empty((P, int(koff[-1])), np.float16)
    for g in range(NB):
        sel = slice(g * P, (g + 1) * P)
        lo, hi = wins[g]
        w = hi - lo
        pinv = np.linalg.pinv(_B[:, lo:hi])              # [w, S]
        C = np.cos(np.outer(beta[sel], i_))
        Sn = np.sin(np.outer(beta[sel], i_))
        ko = int(koff[g])
        kmat[:, ko:ko + w] = (C @ pinv.T).astype(np.float16)
        kmat[:, ko + w:ko + 2 * w] = (-(Sn @ pinv.T)).astype(np.float16)

    bmat = np.zeros((P, 2 * SP), np.float16)
    bmat[:, 0:S] = _B.T.astype(np.float16)                      # B[f, i]
    bmat[:, SP:SP + S] = ((_B * (i_ / S)[:, None]).T / SD).astype(np.float16)

    # envelope at all global chunk nodes and phase at all chunk starts
    nodes = np.arange(N_CORES * NCH + 1) * S / FS               # [401]
    G = amp[:, None] * np.exp(
        -0.5 * ((nodes[None, :] - tau[:, None]) / sigma[:, None]) ** 2)
    starts = np.arange(N_CORES * NCH) * S / FS                  # [400]
    dt = starts[None, :] - tau[:, None]
    ph = 2 * np.pi * omega[:, None] * dt + gam[:, None] * dt * dt + phi[:, None]
    ca = np.cos(ph)
    sa = np.sin(ph)
    e0 = G[:, :-1]
    de = G[:, 1:] - G[:, :-1]
    Ec0 = (e0 * ca).astype(np.float16).view(np.uint16)
    Es0 = (e0 * sa).astype(np.float16).view(np.uint16)
    Dc = (de * ca * SD).astype(ml_dtypes.float8_e4m3fn).view(np.uint8)
    Ds = (de * sa * SD).astype(ml_dtypes.float8_e4m3fn).view(np.uint8)

    offs = np.concatenate([[0], np.cumsum(PCS)]).astype(int)
    in_maps = []
    for c in range(N_CORES):
        m = {"kmat": kmat, "bmat": bmat}
        for p, nch in enumerate(PCS):
            cs = slice(c * NCH + int(offs[p]), c * NCH + int(offs[p]) + nch)
            em = np.empty((P, NB * 3 * nch), np.uint16)
            for g in range(NB):
                sel = slice(g * P, (g + 1) * P)
                o = g * 3 * nch
                em[:, o:o + nch] = Ec0[sel, cs]
                em[:, o + nch:o + 2 * nch] = Es0[sel, cs]
                pk = np.concatenate([Dc[sel, cs], Ds[sel, cs]], axis=1)
                em[:, o + 2 * nch:o + 3 * nch] = \
                    np.ascontiguousarray(pk).view(np.uint16)
            m[f"emat{p}"] = np.ascontiguousarray(em)
        in_maps.append(m)
    return wins, in_maps


def kernel(amplitude_logit, tau, omega_logit, sigma_logit, phi_vector, gamma, t):
    wins, in_maps = _prepare(amplitude_logit, tau, omega_logit, sigma_logit,
                             phi_vector, gamma)
    if wins not in _cache:
        _cache[wins] = _build_program(wins)
    nc = _cache[wins]
    res = run_bass_kernel_spmd(nc, in_maps, list(range(N_CORES)))
    total = np.empty(T, dtype=np.float32)
    for c, r in enumerate(res.results):
        wv = r["wave"].astype(np.float32)          # [128, NCH]
        total[c * TC:(c + 1) * TC] = wv[:S].T.ravel()
    return total
